# revision 8
# baseline (speedup 1.0000x reference)
"""Multi-head attention Bass kernel for Trainium2, 8-core SPMD. v2.

Problem: B=2, S=4096, D=512, H=8 heads, head_dim=64, fp32 in/out.
Sharding: batch x query-slice (core c -> batch c//4, query rows
(c%4)*1024 .. +1024). Each core computes all 8 heads for its query
slice against the full key/value sequence of its batch; outputs
partition disjointly so no cross-core reduction is needed.

v2 changes vs baseline (622us -> target ~230us):
  1. Score matmuls (ST) emitted head-parity-paired: consecutive
     matmuls alternate PE row tiles (partitions 0-63 even head /
     64-127 odd head, tile_position inferred), which the PE runs
     concurrently: measured 211us -> 76us (pair) / 49us (pair,4buf).
  2. Softmax exp split across engines: even heads on ACT (table
     exp), odd heads on a custom DVE op (int16 Schraudolph with
     |r|-quadratic correction; rms 0.19% elem error, validated
     bit-exact vs numpy model on HW). W_k is pre-scaled by
     SCORE_PRESCALE on host so scores arrive in 2^(t/1024) units.
  3. x loads cast-DMA fp32->fp16 then xbar DMA-transpose (HWDGE)
     instead of PE transpose + ACT copy; frees PE/ACT in load phase.
  4. xk/xv DMA-interleaved; K/V projections chase the DMA wave,
     head-pair-0 attention chases the projections (subtile deps).
"""

import numpy as np

B, S, D, H, HD = 2, 4096, 512, 8, 64
N_CORES = 8
QSL = S * B // N_CORES  # 1024 query rows per core

# exp-op constants (fit in fit3; validated on HW: round-to-nearest int16)
SCORE_PRESCALE = float(0.125 * np.log2(np.e) * 1024)  # folded into W_k
EXP_C1 = -0.33251070070853966
EXP_C2 = 0.00031153687147665826
EXP_M2 = float(2.0 ** 33)
EXP_K15 = float(15 * 1024)
ACT_SCALE = float(0.125 / SCORE_PRESCALE)  # undo prescale on the ACT path

_CACHE = {}


def _register_exp_op():
    """Register the custom DVE exp op (idempotent)."""
    if "exp_op" in _CACHE:
        return _CACHE["exp_op"]
    from concourse import dve_ops as DO
    from concourse.dve_spec import Spec, Src0, C0, C1, C2, C3, Bin, lower
    from concourse.dve_uop import AluOp, DveOpSpec
    from concourse.dve_ops import DveOp, _spill_c3_to_src1

    y3 = Src0 + C3          # C3 spill via in1: 15360
    u = y3 + C2             # imm2: 2^33
    n = u - C2              # round1024(y3)
    ab = Bin(AluOp.ABSOLUTE_DIFF, y3, n)
    m1 = ab * C1
    m2 = m1 + C0
    psi = ab * m2
    t = y3 + psi
    body = _spill_c3_to_src1(t)

    def ref(in0, in1, s0, s1, imm2):
        y3 = (in0 + in1).astype(np.float32)
        u = (y3 + np.float32(imm2)).astype(np.float32)
        n = (u - np.float32(imm2)).astype(np.float32)
        ab = np.abs(y3 - n).astype(np.float32)
        return (y3 + ab * (ab * np.float32(s1) + np.float32(s0))).astype(
            np.float32)

    spec = Spec(body=body, reference=ref)
    for op in DO.OPS:
        if op.name == "EXP16_ANT":
            _CACHE["exp_op"] = op
            return op
    row = DO._CUSTOM_DVE_ROW_BASE + len(DO.OPS)
    shas = {}
    for ver in ("v3", "v4"):
        d = DveOpSpec(name="EXP16_ANT", opcode=row,
                      uops=lower(spec, ver=ver), rd1_en=True)
        shas[ver] = d.sha(ver)
    op = DveOp("EXP16_ANT", spec, subdim=False, uops_sha=shas)
    DO.OPS.append(op)
    DO._SUB_OPCODE_FOR_NAME[op.name] = row
    DO.CUSTOM_DVE_SPECS[op.name] = op.spec
    _CACHE["exp_op"] = op
    return op


def build_nc(s=S, qsl=QSL, debug=False, reps=1, phases="all"):
    """phases: "all" | subsets for timing isolation:
    "load" x loads + transposes only, "proj" loads + projections,
    "st" score matmuls only (memset inputs), "stexp" +exp,
    "attn" full attention + epilogue (memset inputs)."""
    import contextlib
    import concourse.bacc as bacc
    import concourse.tile as tile
    import concourse.mybir as mybir

    exp_op = _register_exp_op()

    do_load = phases in ("all", "load", "proj")
    do_proj = phases in ("all", "proj")
    do_st = phases in ("all", "attn", "st", "stexp")
    do_exp = phases in ("all", "attn", "stexp")
    do_pv = phases in ("all", "attn")

    f32 = mybir.dt.float32
    f16 = mybir.dt.float16
    i16 = mybir.dt.int16
    Exp = mybir.ActivationFunctionType.Exp
    mult = mybir.AluOpType.mult

    KB = s // 128        # 32 k blocks
    QB = qsl // 128      # 8 q blocks
    NJ = D // 128        # 4 din chunks
    H2 = H // 2          # 4 head pairs
    QS = 512             # q-span per matmul (PSUM bank limit)
    NQS = qsl // QS

    nc = bacc.Bacc("TRN2", target_bir_lowering=False, debug=debug,
                   num_devices=N_CORES)
    xq_d = nc.dram_tensor("xq", [qsl, D], f32, kind="ExternalInput")
    xk_d = nc.dram_tensor("xk", [s, D], f32, kind="ExternalInput")
    xv_d = nc.dram_tensor("xv", [s, D], f32, kind="ExternalInput")
    wq_d = nc.dram_tensor("wq", [D, D], f32, kind="ExternalInput")
    wk_d = nc.dram_tensor("wk", [D, D], f32, kind="ExternalInput")
    wv_d = nc.dram_tensor("wv", [D, D], f32, kind="ExternalInput")
    wo_d = nc.dram_tensor("wo", [D, D], f32, kind="ExternalInput")
    out_d = nc.dram_tensor("out", [qsl, D], f32, kind="ExternalOutput")

    with tile.TileContext(nc) as tc:
        loop = tc.For_i(0, reps) if reps > 1 else contextlib.nullcontext()
        with loop, (
            tc.tile_pool(name="const", bufs=1)) as cpool, (
            tc.tile_pool(name="persist", bufs=1)) as pers, (
            tc.tile_pool(name="xcast", bufs=4)) as xcast, (
            tc.tile_pool(name="ptpool", bufs=4)) as ptpool, (
            tc.tile_pool(name="ostage", bufs=2)) as ostage:

            ones64 = cpool.tile([1, 64], f16, name="ones64")
            nc.gpsimd.memset(ones64[:], 1.0)
            b15 = cpool.tile([128, 1], f32, name="b15")
            nc.gpsimd.memset(b15[:], EXP_K15)

            # ---- weights: gpsimd cast-DMA fp32 -> fp16, chunked ----------
            w16 = {}
            for nm, wd in (("wq", wq_d), ("wk", wk_d), ("wv", wv_d),
                           ("wo", wo_d)):
                wt = pers.tile([128, NJ, D], f16, name=f"{nm}16")
                if do_load:
                    nc.gpsimd.dma_start(
                        wt[:], wd.rearrange("(j p) d -> p j d", p=128))
                w16[nm] = wt

            # ---- persistent activations ---------------------------------
            KT = pers.tile([128, NJ, s], f16, name="KT")
            QT = pers.tile([128, NJ, qsl], f16, name="QT")
            Vp = pers.tile([128, KB, H * 65], f16, name="Vp")
            Vp_v = Vp.rearrange("p k (h c) -> p k h c", c=65)
            otz2 = pers.tile([128, H2, qsl], f16, name="otz2")
            rz16f = pers.tile([1, H, qsl], f16, name="rz16f")

            # ones columns of V' (softmax denominator trick)
            nc.gpsimd.memset(Vp_v[:, :, :, 64:65], 1.0)

            if do_st and not do_proj:
                # timing-only variants: give KT/QT/Vp defined contents
                nc.gpsimd.memset(KT[:], 0.001)
                nc.gpsimd.memset(QT[:], 0.001)
                nc.gpsimd.memset(Vp_v[:, :, :, 0:64], 0.001)

            def load_block(xd, xT, i):
                """cast-DMA one 128-token block, xbar-transpose to xT."""
                xc = xcast.tile([128, D], f16, name=f"xc_{xd.name}_{i}",
                                tag="xc")
                nc.gpsimd.dma_start(xc[:], xd[i * 128:(i + 1) * 128, :])
                nc.sync.dma_start(xT[:, :, i * 128:(i + 1) * 128], xc[:],
                                  transpose=True)

            # ---- load + projection phase (pppool: 2 PSUM banks) ---------
            with (
                tc.tile_pool(name="xT", bufs=1) as xTp,
                tc.tile_pool(name="ppp", bufs=2, space="PSUM") as pppool,
            ):
                xqT = xTp.tile([128, NJ, qsl], f16, name="xqT", tag="xqT")
                xkT = xTp.tile([128, NJ, s], f16, name="xkT", tag="xkT")
                xvT = xTp.tile([128, NJ, s], f16, name="xvT", tag="xvT")

                def q_pipeline():
                    if not do_load:
                        return
                    for i in range(QB):
                        load_block(xq_d, xqT, i)
                    for m in range(NJ if do_proj else 0):
                        for ks in range(NQS):
                            pp = pppool.tile([128, 512], f32,
                                             name=f"qpp_{m}_{ks}", tag="pp")
                            for j in range(NJ):
                                nc.tensor.matmul(
                                    pp[:, 0:QS],
                                    w16["wq"][:, j, m * 128:(m + 1) * 128],
                                    xqT[:, j, ks * QS:(ks + 1) * QS],
                                    start=(j == 0), stop=(j == NJ - 1))
                            nc.scalar.copy(QT[:, m, ks * QS:(ks + 1) * QS],
                                           pp[:, 0:QS])

                def kv_load(i):
                    load_block(xk_d, xkT, i)
                    load_block(xv_d, xvT, i)

                def k_proj(ks):
                    """project k-token span [ks*512, +512) for all dout."""
                    for m in range(NJ):
                        pp = pppool.tile([128, 512], f32,
                                         name=f"kpp_{m}_{ks}", tag="pp")
                        for j in range(NJ):
                            nc.tensor.matmul(
                                pp[:], w16["wk"][:, j, m * 128:(m + 1) * 128],
                                xkT[:, j, ks * 512:(ks + 1) * 512],
                                start=(j == 0), stop=(j == NJ - 1))
                        nc.scalar.copy(KT[:, m, ks * 512:(ks + 1) * 512],
                                       pp[:])

                def v_proj(i):
                    pp = pppool.tile([128, D], f32, name=f"vpp_{i}", tag="pp")
                    for j in range(NJ):
                        nc.tensor.matmul(pp[:],
                                         xvT[:, j, i * 128:(i + 1) * 128],
                                         w16["wv"][:, j, :],
                                         start=(j == 0), stop=(j == NJ - 1))
                    nc.scalar.copy(Vp_v[:, i, :, 0:64],
                                   pp.rearrange("p (h c) -> p h c", c=64))

                # ---- emission: loads + projections ----------------------
                q_pipeline()
                if do_load:
                    # interleave xk/xv loads; projections chase per-span
                    for i in range(KB):
                        kv_load(i)
                        if do_proj and i % 4 == 3:
                            k_proj(i // 4)
                        if do_proj:
                            v_proj(i)

            # ---- attention phase (stp 4 + otp 4 PSUM banks) -------------
            with (
                tc.tile_pool(name="stp", bufs=2, space="PSUM") as stpool,
                tc.tile_pool(name="otp", bufs=2, space="PSUM") as otpool,
                tc.tile_pool(name="rzp", bufs=2) as rzpool,
            ):
                st_of = {}
                pt_of = {}
                ot_ps = {}

                def emit_st(m, i):
                    """paired STs: even head rows 0-63, odd rows 64-127."""
                    sts = []
                    for par in (0, 1):
                        st = stpool.tile([128, qsl], f32,
                                         name=f"st_{m}_{i}_{par}", tag="st")
                        sts.append(st)
                    for q0 in range(NQS):
                        for par, st in enumerate(sts):
                            po = par * 64
                            nc.tensor.matmul(
                                st[:, q0 * QS:(q0 + 1) * QS],
                                KT[po:po + 64, m, i * 128:(i + 1) * 128],
                                QT[po:po + 64, m, q0 * QS:(q0 + 1) * QS],
                                start=True, stop=True)
                    st_of[(m, i)] = sts

                def emit_exp(m, i):
                    if not do_exp:
                        st_of.pop((m, i))
                        return
                    sts = st_of.pop((m, i))
                    pts = []
                    for par, st in enumerate(sts):
                        pt = ptpool.tile([128, qsl], f16,
                                         name=f"pt_{m}_{i}_{par}", tag="pt")
                        if par == 0:
                            nc.scalar.activation(pt[:], st[:], Exp,
                                                 scale=ACT_SCALE)
                        else:
                            nc.vector._custom_dve(
                                exp_op, out=pt[:].bitcast(i16), in0=st[:],
                                in1=b15[:], s0=EXP_C1, s1=EXP_C2, imm2=EXP_M2)
                        pts.append(pt)
                    pt_of[(m, i)] = pts

                def emit_pv(m, i):
                    if (m, i) not in pt_of:
                        return
                    pts = pt_of.pop((m, i))
                    if i == 0:
                        ot_ps[m] = [otpool.tile([128, qsl], f32,
                                                name=f"ot_{m}_{par}",
                                                tag="ot")
                                    for par in (0, 1)]
                    for q0 in range(NQS):
                        for par, pt in enumerate(pts):
                            h = 2 * m + par
                            nc.tensor.matmul(
                                ot_ps[m][par][0:65, q0 * QS:(q0 + 1) * QS],
                                Vp_v[:, i, h, :],
                                pt[:, q0 * QS:(q0 + 1) * QS],
                                start=(i == 0), stop=(i == KB - 1))
                    if i == KB - 1:
                        ots = ot_ps.pop(m)
                        for par, ot in enumerate(ots):
                            h = 2 * m + par
                            po2 = par * 64
                            nc.vector.tensor_copy(otz2[po2:po2 + 64, m, :],
                                                  ot[0:64, :])
                            rzt = rzpool.tile([1, qsl], f32,
                                              name=f"rzt_{h}", tag="rzt")
                            nc.vector.reciprocal(rzt[:], ot[64:65, :])
                            nc.vector.tensor_copy(rz16f[0:1, h, :], rzt[:])

                if do_st:
                    # PV(idx-1) first so PE has work while exp(idx-1)
                    # frees the st banks that ST(idx) needs (stp bufs=2
                    # holds one head-pair slot; 8-bank PSUM limit)
                    seq = [(m, i) for m in range(H2) for i in range(KB)]
                    emit_st(*seq[0])
                    emit_exp(*seq[0])
                    for idx in range(1, len(seq)):
                        if do_pv:
                            emit_pv(*seq[idx - 1])
                        emit_st(*seq[idx])
                        emit_exp(*seq[idx])
                    if do_pv:
                        emit_pv(*seq[-1])

            # ---- normalize + output projection --------------------------
            if do_pv:
                    with tc.tile_pool(name="fgp", bufs=2,
                                      space="PSUM") as fgpool:
                        for m in range(H2):
                            rzb = fgpool.tile([128, qsl], f32,
                                              name=f"rzb_{m}", tag="rzb")
                            for half in (0, 1):
                                h = 2 * m + half
                                for q0 in range(NQS):
                                    nc.tensor.matmul(
                                        rzb[half * 64:half * 64 + 64,
                                            q0 * QS:(q0 + 1) * QS],
                                        ones64[:],
                                        rz16f[0:1, h, q0 * QS:(q0 + 1) * QS],
                                        start=True, stop=True)
                            nc.vector.tensor_tensor(out=otz2[:, m, :],
                                                    in0=otz2[:, m, :],
                                                    in1=rzb[:], op=mult)
                        for qb in range(QB):
                            pf = fgpool.tile([128, D], f32, name=f"pf_{qb}",
                                             tag="pf")
                            for m in range(H2):
                                nc.tensor.matmul(
                                    pf[:], otz2[:, m, qb * 128:(qb + 1) * 128],
                                    w16["wo"][:, m, :],
                                    start=(m == 0), stop=(m == H2 - 1))
                            ob = ostage.tile([128, D], f32, name=f"ob_{qb}",
                                             tag="ob")
                            nc.vector.tensor_copy(ob[:], pf[:])
                            nc.sync.dma_start(out_d[qb * 128:(qb + 1) * 128, :],
                                              ob[:])

    nc.finalize()
    return nc


def _in_maps(x_q, x_k, x_v, W_q, W_k, W_v, W_o):
    """Slice full inputs into per-core input maps (batch x q-slice).

    W_k is pre-scaled so scores arrive in 2^(t/1024) units for the
    DVE exp path (ACT path undoes it via its free affine scale)."""
    Wk_s = np.ascontiguousarray(W_k * np.float32(SCORE_PRESCALE))
    qpb = N_CORES // B  # cores per batch
    maps = []
    for c in range(N_CORES):
        b, qi = c // qpb, c % qpb
        maps.append({
            "xq": np.ascontiguousarray(x_q[b, qi * QSL:(qi + 1) * QSL, :]),
            "xk": np.ascontiguousarray(x_k[b]),
            "xv": np.ascontiguousarray(x_v[b]),
            "wq": W_q, "wk": Wk_s, "wv": W_v, "wo": W_o,
        })
    return maps


def kernel(x_q, x_k, x_v, mask, W_q, b_q, W_k, b_k, W_v, b_v, W_o, b_o):
    """Full-input entry point: shard across 8 cores, run, gather.

    The compiled SPMD executable is cached in-process, so repeat calls
    pay only input transfer + device execution."""
    import jax
    from jax.sharding import Mesh, PartitionSpec, NamedSharding
    from jax.experimental.shard_map import shard_map
    import concourse.mybir as mybir
    from concourse import bass2jax

    if "runner" not in _CACHE:
        nc = build_nc()
        bass2jax.install_neuronx_cc_hook()
        pname = nc.partition_id_tensor.name if nc.partition_id_tensor else None
        in_names, out_names, out_avals, zero_outs = [], [], [], []
        for alloc in nc.m.functions[0].allocations:
            if not isinstance(alloc, mybir.MemoryLocationSet):
                continue
            name = alloc.memorylocations[0].name
            if alloc.kind == "ExternalInput":
                if name != pname:
                    in_names.append(name)
            elif alloc.kind == "ExternalOutput":
                shape = tuple(alloc.tensor_shape)
                dtype = mybir.dt.np(alloc.dtype)
                out_names.append(name)
                out_avals.append(jax.core.ShapedArray(shape, dtype))
                zero_outs.append(np.zeros(shape, dtype))
        n_params = len(in_names)
        all_in = list(in_names) + list(out_names)
        if pname is not None:
            all_in.append(pname)

        def _body(*args):
            ops = list(args)
            if pname is not None:
                ops.append(bass2jax.partition_id_tensor())
            return tuple(bass2jax._bass_exec_p.bind(
                *ops,
                out_avals=tuple(out_avals),
                in_names=tuple(all_in),
                out_names=tuple(out_names),
                lowering_input_output_aliases=(),
                sim_require_finite=False,
                sim_require_nnan=False,
                nc=nc,
            ))

        devices = jax.devices()[:N_CORES]
        mesh = Mesh(np.asarray(devices), ("core",))
        specs = (PartitionSpec("core"),)
        fn = jax.jit(
            shard_map(_body, mesh=mesh,
                      in_specs=specs * (n_params + len(out_names)),
                      out_specs=specs * len(out_names), check_rep=False),
            keep_unused=True,
        )
        sh = NamedSharding(mesh, PartitionSpec("core"))
        zero_dev = [jax.device_put(
            np.zeros((N_CORES * z.shape[0], *z.shape[1:]), z.dtype), sh)
            for z in zero_outs]
        _CACHE["runner"] = (fn, in_names, zero_dev, sh)
    fn, in_names, zero_dev, sh = _CACHE["runner"]

    f32 = np.float32
    maps = _in_maps(np.asarray(x_q, f32), np.asarray(x_k, f32),
                    np.asarray(x_v, f32), np.asarray(W_q, f32),
                    np.asarray(W_k, f32), np.asarray(W_v, f32),
                    np.asarray(W_o, f32))
    import jax as _jax
    concat_in = [np.concatenate([maps[c][n] for c in range(N_CORES)])
                 for n in in_names]
    dev_in = [_jax.device_put(a, sh) for a in concat_in]
    outs = fn(*dev_in, *zero_dev)
    res = np.asarray(outs[0]).reshape(N_CORES, QSL, D)

    out = np.empty((B, S, D), np.float32)
    qpb = N_CORES // B
    for c in range(N_CORES):
        b, qi = c // qpb, c % qpb
        out[b, qi * QSL:(qi + 1) * QSL, :] = res[c]
    return out


# revision 14
# speedup vs baseline: 1.6837x; 1.6837x over previous
"""Multi-head attention Bass kernel for Trainium2, 8-core SPMD. v2.

Problem: B=2, S=4096, D=512, H=8 heads, head_dim=64, fp32 in/out.
Sharding: batch x query-slice (core c -> batch c//4, query rows
(c%4)*1024 .. +1024). Each core computes all 8 heads for its query
slice against the full key/value sequence of its batch; outputs
partition disjointly so no cross-core reduction is needed.

v2 changes vs baseline (622us -> target ~230us):
  1. Score matmuls (ST) emitted head-parity-paired: consecutive
     matmuls alternate PE row tiles (partitions 0-63 even head /
     64-127 odd head, tile_position inferred), which the PE runs
     concurrently: measured 211us -> 76us (pair) / 49us (pair,4buf).
  2. Softmax exp split across engines: even heads on ACT (table
     exp), odd heads on a custom DVE op (int16 Schraudolph with
     |r|-quadratic correction; rms 0.19% elem error, validated
     bit-exact vs numpy model on HW). W_k is pre-scaled by
     SCORE_PRESCALE on host so scores arrive in 2^(t/1024) units.
  3. x loads cast-DMA fp32->fp16 then xbar DMA-transpose (HWDGE)
     instead of PE transpose + ACT copy; frees PE/ACT in load phase.
  4. xk/xv DMA-interleaved; K/V projections chase the DMA wave,
     head-pair-0 attention chases the projections (subtile deps).
"""

import numpy as np

B, S, D, H, HD = 2, 4096, 512, 8, 64
N_CORES = 8
QSL = S * B // N_CORES  # 1024 query rows per core

# exp-op constants (fit in fit3; validated on HW: round-to-nearest int16)
SCORE_PRESCALE = float(0.125 * np.log2(np.e) * 1024)  # folded into W_k
EXP_C1 = -0.33251070070853966
EXP_C2 = 0.00031153687147665826
EXP_M2 = float(2.0 ** 33)
EXP_K15 = float(15 * 1024)
ACT_SCALE = float(0.125 / SCORE_PRESCALE)  # undo prescale on the ACT path

_CACHE = {}


def _register_exp_op():
    """Register the custom DVE exp op (idempotent)."""
    if "exp_op" in _CACHE:
        return _CACHE["exp_op"]
    from concourse import dve_ops as DO
    from concourse.dve_spec import Spec, Src0, C0, C1, C2, C3, Bin, lower
    from concourse.dve_uop import AluOp, DveOpSpec
    from concourse.dve_ops import DveOp, _spill_c3_to_src1

    y3 = Src0 + C3          # C3 spill via in1: 15360
    u = y3 + C2             # imm2: 2^33
    n = u - C2              # round1024(y3)
    ab = Bin(AluOp.ABSOLUTE_DIFF, y3, n)
    m1 = ab * C1
    m2 = m1 + C0
    psi = ab * m2
    t = y3 + psi
    body = _spill_c3_to_src1(t)

    def ref(in0, in1, s0, s1, imm2):
        y3 = (in0 + in1).astype(np.float32)
        u = (y3 + np.float32(imm2)).astype(np.float32)
        n = (u - np.float32(imm2)).astype(np.float32)
        ab = np.abs(y3 - n).astype(np.float32)
        return (y3 + ab * (ab * np.float32(s1) + np.float32(s0))).astype(
            np.float32)

    spec = Spec(body=body, reference=ref)
    for op in DO.OPS:
        if op.name == "EXP16_ANT":
            _CACHE["exp_op"] = op
            return op
    row = DO._CUSTOM_DVE_ROW_BASE + len(DO.OPS)
    shas = {}
    for ver in ("v3", "v4"):
        d = DveOpSpec(name="EXP16_ANT", opcode=row,
                      uops=lower(spec, ver=ver), rd1_en=True)
        shas[ver] = d.sha(ver)
    op = DveOp("EXP16_ANT", spec, subdim=False, uops_sha=shas)
    DO.OPS.append(op)
    DO._SUB_OPCODE_FOR_NAME[op.name] = row
    DO.CUSTOM_DVE_SPECS[op.name] = op.spec
    _CACHE["exp_op"] = op
    return op


def build_nc(s=S, qsl=QSL, debug=False, reps=1, phases="all"):
    """phases: "all" | subsets for timing isolation:
    "load" x loads + transposes only, "proj" loads + projections,
    "st" score matmuls only (memset inputs), "stexp" +exp,
    "attn" full attention + epilogue (memset inputs)."""
    import contextlib
    import concourse.bacc as bacc
    import concourse.tile as tile
    import concourse.mybir as mybir
    from concourse.masks import make_identity

    exp_op = _register_exp_op()

    do_load = phases in ("all", "load", "proj")
    do_proj = phases in ("all", "proj")
    do_st = phases in ("all", "attn", "st", "stexp")
    do_exp = phases in ("all", "attn", "stexp")
    do_pv = phases in ("all", "attn")

    f32 = mybir.dt.float32
    f16 = mybir.dt.float16
    i16 = mybir.dt.int16
    Exp = mybir.ActivationFunctionType.Exp
    mult = mybir.AluOpType.mult

    KB = s // 128        # 32 k blocks
    QB = qsl // 128      # 8 q blocks
    NJ = D // 128        # 4 din chunks
    H2 = H // 2          # 4 head pairs
    QS = 512             # q-span per matmul (PSUM bank limit)
    NQS = qsl // QS

    nc = bacc.Bacc("TRN2", target_bir_lowering=False, debug=debug,
                   num_devices=N_CORES)
    xq_d = nc.dram_tensor("xq", [qsl, D], f32, kind="ExternalInput")
    xk_d = nc.dram_tensor("xk", [s, D], f32, kind="ExternalInput")
    xv_d = nc.dram_tensor("xv", [s, D], f32, kind="ExternalInput")
    wq_d = nc.dram_tensor("wq", [D, D], f32, kind="ExternalInput")
    wk_d = nc.dram_tensor("wk", [D, D], f32, kind="ExternalInput")
    wv_d = nc.dram_tensor("wv", [D, D], f32, kind="ExternalInput")
    wo_d = nc.dram_tensor("wo", [D, D], f32, kind="ExternalInput")
    out_d = nc.dram_tensor("out", [qsl, D], f32, kind="ExternalOutput")

    with tile.TileContext(nc) as tc:
        loop = tc.For_i(0, reps) if reps > 1 else contextlib.nullcontext()
        with loop, (
            tc.tile_pool(name="const", bufs=1)) as cpool, (
            tc.tile_pool(name="persist", bufs=1)) as pers, (
            tc.tile_pool(name="xcast", bufs=4)) as xcast, (
            tc.tile_pool(name="ptpool", bufs=8)) as ptpool, (
            tc.tile_pool(name="ostage", bufs=2)) as ostage:

            ones64 = cpool.tile([1, 64], f16, name="ones64")
            nc.gpsimd.memset(ones64[:], 1.0)
            b15 = cpool.tile([128, 1], f32, name="b15")
            nc.gpsimd.memset(b15[:], EXP_K15)
            ident = cpool.tile([128, 128], f16, name="ident")
            make_identity(nc, ident)

            # ---- weights: gpsimd cast-DMA fp32 -> fp16, chunked ----------
            w16 = {}
            for nm, wd in (("wq", wq_d), ("wk", wk_d), ("wv", wv_d),
                           ("wo", wo_d)):
                wt = pers.tile([128, NJ, D], f16, name=f"{nm}16")
                nc.gpsimd.dma_start(
                    wt[:], wd.rearrange("(j p) d -> p j d", p=128))
                w16[nm] = wt

            # ---- persistent activations ---------------------------------
            KT = pers.tile([128, NJ, s], f16, name="KT")
            QT = pers.tile([128, NJ, qsl], f16, name="QT")
            Vp = pers.tile([128, KB, H * 65], f16, name="Vp")
            Vp_v = Vp.rearrange("p k (h c) -> p k h c", c=65)
            otz2 = pers.tile([128, H2, qsl], f16, name="otz2")
            rz16f = pers.tile([1, H, qsl], f16, name="rz16f")

            # ones columns of V' (softmax denominator trick)
            nc.gpsimd.memset(Vp_v[:, :, :, 64:65], 1.0)

            if do_st and not do_proj:
                # timing-only variants: give KT/QT/Vp defined contents
                nc.gpsimd.memset(KT[:], 0.001)
                nc.gpsimd.memset(QT[:], 0.001)
                nc.gpsimd.memset(Vp_v[:, :, :, 0:64], 0.001)

            # ---- load + projection phase (ppp 2 + tpp 2 PSUM banks) -----
            with (
                tc.tile_pool(name="xT", bufs=1) as xTp,
                tc.tile_pool(name="ppp", bufs=2, space="PSUM") as pppool,
                tc.tile_pool(name="tpp", bufs=2, space="PSUM") as tppool,
            ):

                def load_block(xd, xT, i):
                    """cast-DMA block, PE transpose, ACT evict (baseline)."""
                    xc = xcast.tile([128, D], f16, name=f"xc_{xd.name}_{i}",
                                    tag="xc")
                    nc.gpsimd.dma_start(xc[:], xd[i * 128:(i + 1) * 128, :])
                    tp = tppool.tile([128, D], f16,
                                     name=f"tp_{xd.name}_{i}", tag="tp")
                    for j in range(NJ):
                        nc.tensor.transpose(tp[:, j * 128:(j + 1) * 128],
                                            xc[:, j * 128:(j + 1) * 128],
                                            ident[:])
                    nc.scalar.copy(
                        xT[:, :, i * 128:(i + 1) * 128],
                        tp.rearrange("p (j c) -> p j c", j=NJ))
                xqT = xTp.tile([128, NJ, qsl], f16, name="xqT", tag="xqT")
                xkT = xTp.tile([128, NJ, s], f16, name="xkT", tag="xkT")
                xvT = xTp.tile([128, NJ, s], f16, name="xvT", tag="xvT")

                def q_pipeline():
                    if not do_load:
                        return
                    for i in range(QB):
                        load_block(xq_d, xqT, i)
                    for m in range(NJ if do_proj else 0):
                        for ks in range(NQS):
                            pp = pppool.tile([128, 512], f32,
                                             name=f"qpp_{m}_{ks}", tag="pp")
                            for j in range(NJ):
                                nc.tensor.matmul(
                                    pp[:, 0:QS],
                                    w16["wq"][:, j, m * 128:(m + 1) * 128],
                                    xqT[:, j, ks * QS:(ks + 1) * QS],
                                    start=(j == 0), stop=(j == NJ - 1))
                            nc.scalar.copy(QT[:, m, ks * QS:(ks + 1) * QS],
                                           pp[:, 0:QS])

                def kv_load(i):
                    load_block(xk_d, xkT, i)
                    load_block(xv_d, xvT, i)

                def k_proj(ks):
                    """project k-token span [ks*512, +512) for all dout."""
                    for m in range(NJ):
                        pp = pppool.tile([128, 512], f32,
                                         name=f"kpp_{m}_{ks}", tag="pp")
                        for j in range(NJ):
                            nc.tensor.matmul(
                                pp[:], w16["wk"][:, j, m * 128:(m + 1) * 128],
                                xkT[:, j, ks * 512:(ks + 1) * 512],
                                start=(j == 0), stop=(j == NJ - 1))
                        nc.scalar.copy(KT[:, m, ks * 512:(ks + 1) * 512],
                                       pp[:])

                def v_proj(i):
                    pp = pppool.tile([128, D], f32, name=f"vpp_{i}", tag="pp")
                    for j in range(NJ):
                        nc.tensor.matmul(pp[:],
                                         xvT[:, j, i * 128:(i + 1) * 128],
                                         w16["wv"][:, j, :],
                                         start=(j == 0), stop=(j == NJ - 1))
                    nc.scalar.copy(Vp_v[:, i, :, 0:64],
                                   pp.rearrange("p (h c) -> p h c", c=64))

                # ---- emission: loads + projections ----------------------
                q_pipeline()
                if do_load:
                    # interleave xk/xv loads; projections chase per-span
                    for i in range(KB):
                        kv_load(i)
                        if do_proj and i % 4 == 3:
                            k_proj(i // 4)
                        if do_proj:
                            v_proj(i)

            # ---- attention phase (stp 4 + otp 4 PSUM banks) -------------
            with (
                tc.tile_pool(name="stp", bufs=4, space="PSUM") as stpool,
                tc.tile_pool(name="otp", bufs=2, space="PSUM") as otpool,
                tc.tile_pool(name="rzp", bufs=2) as rzpool,
            ):
                st_of = {}
                pt_of = {}
                ot_ps = {}

                def emit_st(m, i):
                    """4 span-MMs alternating PE row tiles (even head
                    rows 0-63, odd rows 64-127); one st bank each."""
                    sts = {}
                    for q0 in range(NQS):
                        for par in (0, 1):
                            st = stpool.tile([128, QS], f32,
                                             name=f"st_{m}_{i}_{par}_{q0}",
                                             tag="st")
                            po = par * 64
                            nc.tensor.matmul(
                                st[:],
                                KT[po:po + 64, m, i * 128:(i + 1) * 128],
                                QT[po:po + 64, m, q0 * QS:(q0 + 1) * QS],
                                start=True, stop=True)
                            sts[(par, q0)] = st
                    st_of[(m, i)] = sts

                def emit_exp(m, i):
                    if not do_exp:
                        st_of.pop((m, i))
                        return
                    sts = st_of.pop((m, i))
                    pts = {}
                    for (par, q0), st in sts.items():
                        pt = ptpool.tile([128, QS], f16,
                                         name=f"pt_{m}_{i}_{par}_{q0}",
                                         tag="pt")
                        if par == 0:
                            nc.scalar.activation(pt[:], st[:], Exp,
                                                 scale=ACT_SCALE)
                        else:
                            nc.vector._custom_dve(
                                exp_op, out=pt[:].bitcast(i16), in0=st[:],
                                in1=b15[:], s0=EXP_C1, s1=EXP_C2, imm2=EXP_M2)
                        pts[(par, q0)] = pt
                    pt_of[(m, i)] = pts

                def emit_pv(m, i):
                    if (m, i) not in pt_of:
                        return
                    pts = pt_of.pop((m, i))
                    if i == 0:
                        ot_ps[m] = [otpool.tile([128, qsl], f32,
                                                name=f"ot_{m}_{par}",
                                                tag="ot")
                                    for par in (0, 1)]
                    for q0 in range(NQS):
                        for par in (0, 1):
                            h = 2 * m + par
                            nc.tensor.matmul(
                                ot_ps[m][par][0:65, q0 * QS:(q0 + 1) * QS],
                                Vp_v[:, i, h, :],
                                pts[(par, q0)][:],
                                start=(i == 0), stop=(i == KB - 1))
                    if i == KB - 1:
                        ots = ot_ps.pop(m)
                        for par, ot in enumerate(ots):
                            h = 2 * m + par
                            po2 = par * 64
                            nc.vector.tensor_copy(otz2[po2:po2 + 64, m, :],
                                                  ot[0:64, :])
                            rzt = rzpool.tile([1, qsl], f32,
                                              name=f"rzt_{h}", tag="rzt")
                            nc.vector.reciprocal(rzt[:], ot[64:65, :])
                            nc.vector.tensor_copy(rz16f[0:1, h, :], rzt[:])

                if do_st:
                    # st bufs=4 at [128,512] gives one slot of pipeline
                    # slack: ST(idx+1) waits only on exp(idx-1)
                    seq = [(m, i) for m in range(H2) for i in range(KB)]
                    emit_st(*seq[0])
                    emit_exp(*seq[0])
                    for idx in range(1, len(seq)):
                        emit_st(*seq[idx])
                        if do_pv:
                            emit_pv(*seq[idx - 1])
                        emit_exp(*seq[idx])
                    if do_pv:
                        emit_pv(*seq[-1])

            # ---- normalize + output projection --------------------------
            if do_pv:
                    with tc.tile_pool(name="fgp", bufs=2,
                                      space="PSUM") as fgpool:
                        for m in range(H2):
                            rzb = fgpool.tile([128, qsl], f32,
                                              name=f"rzb_{m}", tag="rzb")
                            for half in (0, 1):
                                h = 2 * m + half
                                for q0 in range(NQS):
                                    nc.tensor.matmul(
                                        rzb[half * 64:half * 64 + 64,
                                            q0 * QS:(q0 + 1) * QS],
                                        ones64[:],
                                        rz16f[0:1, h, q0 * QS:(q0 + 1) * QS],
                                        start=True, stop=True)
                            nc.vector.tensor_tensor(out=otz2[:, m, :],
                                                    in0=otz2[:, m, :],
                                                    in1=rzb[:], op=mult)
                        for qb in range(QB):
                            pf = fgpool.tile([128, D], f32, name=f"pf_{qb}",
                                             tag="pf")
                            for m in range(H2):
                                nc.tensor.matmul(
                                    pf[:], otz2[:, m, qb * 128:(qb + 1) * 128],
                                    w16["wo"][:, m, :],
                                    start=(m == 0), stop=(m == H2 - 1))
                            ob = ostage.tile([128, D], f32, name=f"ob_{qb}",
                                             tag="ob")
                            nc.vector.tensor_copy(ob[:], pf[:])
                            nc.sync.dma_start(out_d[qb * 128:(qb + 1) * 128, :],
                                              ob[:])

    nc.finalize()
    return nc


def _in_maps(x_q, x_k, x_v, W_q, W_k, W_v, W_o):
    """Slice full inputs into per-core input maps (batch x q-slice).

    W_k is pre-scaled so scores arrive in 2^(t/1024) units for the
    DVE exp path (ACT path undoes it via its free affine scale)."""
    Wk_s = np.ascontiguousarray(W_k * np.float32(SCORE_PRESCALE))
    qpb = N_CORES // B  # cores per batch
    maps = []
    for c in range(N_CORES):
        b, qi = c // qpb, c % qpb
        maps.append({
            "xq": np.ascontiguousarray(x_q[b, qi * QSL:(qi + 1) * QSL, :]),
            "xk": np.ascontiguousarray(x_k[b]),
            "xv": np.ascontiguousarray(x_v[b]),
            "wq": W_q, "wk": Wk_s, "wv": W_v, "wo": W_o,
        })
    return maps


def kernel(x_q, x_k, x_v, mask, W_q, b_q, W_k, b_k, W_v, b_v, W_o, b_o):
    """Full-input entry point: shard across 8 cores, run, gather.

    The compiled SPMD executable is cached in-process, so repeat calls
    pay only input transfer + device execution."""
    import jax
    from jax.sharding import Mesh, PartitionSpec, NamedSharding
    from jax.experimental.shard_map import shard_map
    import concourse.mybir as mybir
    from concourse import bass2jax

    if "runner" not in _CACHE:
        nc = build_nc()
        bass2jax.install_neuronx_cc_hook()
        pname = nc.partition_id_tensor.name if nc.partition_id_tensor else None
        in_names, out_names, out_avals, zero_outs = [], [], [], []
        for alloc in nc.m.functions[0].allocations:
            if not isinstance(alloc, mybir.MemoryLocationSet):
                continue
            name = alloc.memorylocations[0].name
            if alloc.kind == "ExternalInput":
                if name != pname:
                    in_names.append(name)
            elif alloc.kind == "ExternalOutput":
                shape = tuple(alloc.tensor_shape)
                dtype = mybir.dt.np(alloc.dtype)
                out_names.append(name)
                out_avals.append(jax.core.ShapedArray(shape, dtype))
                zero_outs.append(np.zeros(shape, dtype))
        n_params = len(in_names)
        all_in = list(in_names) + list(out_names)
        if pname is not None:
            all_in.append(pname)

        def _body(*args):
            ops = list(args)
            if pname is not None:
                ops.append(bass2jax.partition_id_tensor())
            return tuple(bass2jax._bass_exec_p.bind(
                *ops,
                out_avals=tuple(out_avals),
                in_names=tuple(all_in),
                out_names=tuple(out_names),
                lowering_input_output_aliases=(),
                sim_require_finite=False,
                sim_require_nnan=False,
                nc=nc,
            ))

        devices = jax.devices()[:N_CORES]
        mesh = Mesh(np.asarray(devices), ("core",))
        specs = (PartitionSpec("core"),)
        fn = jax.jit(
            shard_map(_body, mesh=mesh,
                      in_specs=specs * (n_params + len(out_names)),
                      out_specs=specs * len(out_names), check_rep=False),
            keep_unused=True,
        )
        sh = NamedSharding(mesh, PartitionSpec("core"))
        zero_dev = [jax.device_put(
            np.zeros((N_CORES * z.shape[0], *z.shape[1:]), z.dtype), sh)
            for z in zero_outs]
        _CACHE["runner"] = (fn, in_names, zero_dev, sh)
    fn, in_names, zero_dev, sh = _CACHE["runner"]

    f32 = np.float32
    maps = _in_maps(np.asarray(x_q, f32), np.asarray(x_k, f32),
                    np.asarray(x_v, f32), np.asarray(W_q, f32),
                    np.asarray(W_k, f32), np.asarray(W_v, f32),
                    np.asarray(W_o, f32))
    import jax as _jax
    concat_in = [np.concatenate([maps[c][n] for c in range(N_CORES)])
                 for n in in_names]
    dev_in = [_jax.device_put(a, sh) for a in concat_in]
    outs = fn(*dev_in, *zero_dev)
    res = np.asarray(outs[0]).reshape(N_CORES, QSL, D)

    out = np.empty((B, S, D), np.float32)
    qpb = N_CORES // B
    for c in range(N_CORES):
        b, qi = c // qpb, c % qpb
        out[b, qi * QSL:(qi + 1) * QSL, :] = res[c]
    return out
